# revision 7
# baseline (speedup 1.0000x reference)
"""Mode-adaptive linear (MoE soft routing) Trainium2 kernel.

out[b, o] = sum_c weights[b, c] * (inputs[b, :] @ w[c])[o] + (weights @ bias)[b, o]

Strategy: data-parallel shard of the batch across 8 NeuronCores (1024 rows
each); w/bias replicated.  On each core the routing weights are folded into
the transposed input tiles (xs_c = weights[:, c] * X^T in bf16), so all 8
expert matmuls plus the bias matmul accumulate into a single PSUM bank per
128-row batch tile — equivalent to one K=4104 matmul per tile.
"""

import json
import types

import numpy as np

import concourse.bass as bass
import concourse.mybir as mybir
import concourse.tile as tile
from concourse.bass import ts
from concourse.bass_utils import run_bass_kernel_spmd
from concourse.masks import make_identity

N_CORES = 8
B, D_IN, D_OUT, N_CTRL = 8192, 512, 512, 8
B_SHARD = B // N_CORES          # 1024 rows per core
P = 128
N_TILES = B_SHARD // P          # 8 batch tiles per core
KS = D_IN // P                  # 4 K-chunks of 128
F32 = mybir.dt.float32
BF16 = mybir.dt.bfloat16


def _body(nc: bass.Bass, tc: tile.TileContext, x_d, wt_d, w_d, b_d, o_d):
    with (
        tc.tile_pool(name="const", bufs=1) as const_pool,
        tc.tile_pool(name="wstage", bufs=2) as wstage,
        tc.tile_pool(name="xpool", bufs=3) as xpool,
        tc.tile_pool(name="xtpool", bufs=3) as xtpool,
        tc.tile_pool(name="xspool", bufs=2) as xspool,
        tc.tile_pool(name="opool", bufs=3) as opool,
        tc.tile_pool(name="tr_ps", bufs=2, space="PSUM") as tr_psum,
        tc.tile_pool(name="mm_ps", bufs=2, space="PSUM") as mm_psum,
        tc.tile_pool(name="bc_ps", bufs=2, space="PSUM") as bc_psum,
    ):
        identity = const_pool.tile([P, P], F32)
        make_identity(nc, identity)

        # Routing weights transposed to [8, 1024] via one small strided DMA,
        # then zero-padded to 128 partitions in bf16 (rows 8..127 = 0 so it
        # can serve as a K=128 matmul operand).
        wt_t_f32 = const_pool.tile([N_CTRL, B_SHARD], F32)
        wt_pad = const_pool.tile([P, B_SHARD], BF16)
        nc.gpsimd.memset(wt_pad, 0.0)
        WT_CHUNK = 256
        with nc.allow_non_contiguous_dma(
            reason="one-time 32KB routing-weight transpose load"
        ):
            for h in range(B_SHARD // WT_CHUNK):
                nc.sync.dma_start(
                    wt_t_f32[:, ts(h, WT_CHUNK)],
                    wt_d[ts(h, WT_CHUNK), :].rearrange("b c -> c b"),
                )
                nc.vector.tensor_copy(
                    wt_pad[0:N_CTRL, ts(h, WT_CHUNK)], wt_t_f32[:, ts(h, WT_CHUNK)]
                )

        # Bias, zero-padded the same way.
        b_f32 = const_pool.tile([N_CTRL, D_OUT], F32)
        nc.sync.dma_start(b_f32, b_d)
        b_pad = const_pool.tile([P, D_OUT], BF16)
        nc.gpsimd.memset(b_pad, 0.0)
        nc.vector.tensor_copy(b_pad[0:N_CTRL, :], b_f32)

        # Selection matrix: block c has row c all-ones, so
        # matmul(e_pad[:, c], wt_pad) broadcasts wt row c to 128 partitions.
        # e_pad[p, c, m] = 1.0 iff p == c, built via affine_select.
        e_pad = const_pool.tile([P, N_CTRL, P], BF16)
        nc.gpsimd.memset(e_pad, 0.0)
        nc.gpsimd.affine_select(
            out=e_pad,
            in_=e_pad,
            compare_op=mybir.AluOpType.not_equal,
            fill=1.0,
            base=0,
            # predicate: p - c != 0 ? keep in_ (0) : fill (1)
            pattern=[[-1, N_CTRL], [0, P]],
            channel_multiplier=1,
        )

        # Wb[p, c, b] = weights[b, c] for every partition p.
        wb = const_pool.tile([P, N_CTRL, B_SHARD], BF16)
        for c in range(N_CTRL):
            for h in range(B_SHARD // 512):
                bc_ps = bc_psum.tile([P, 512], F32)
                nc.tensor.matmul(
                    bc_ps,
                    lhsT=e_pad[:, c, :],
                    rhs=wt_pad[:, ts(h, 512)],
                    start=True,
                    stop=True,
                )
                nc.scalar.copy(wb[:, c, ts(h, 512)], bc_ps)

        # Expert weights: [128 (i%128), expert, i//128, o] in bf16.
        w_sb = const_pool.tile([P, N_CTRL, KS, D_OUT], BF16)
        for c in range(N_CTRL):
            w_f32 = wstage.tile([P, KS, D_OUT], F32)
            nc.sync.dma_start(w_f32, w_d[c].rearrange("(k p) o -> p k o", p=P))
            if c % 2 == 0:
                nc.vector.tensor_copy(w_sb[:, c], w_f32)
            else:
                nc.scalar.copy(w_sb[:, c], w_f32)

        for t in range(N_TILES):
            x_f32 = xpool.tile([P, D_IN], F32)
            nc.sync.dma_start(x_f32, x_d[ts(t, P), :])

            # X^T for this tile via PE transpose, cast to bf16 on ScalarE.
            tr_ps = tr_psum.tile([P, KS, P], F32)
            xt = xtpool.tile([P, KS, P], BF16)
            for k in range(KS):
                nc.tensor.transpose(tr_ps[:, k, :], x_f32[:, ts(k, P)], identity)
                nc.scalar.copy(xt[:, k, :], tr_ps[:, k, :])

            # Fold routing weights in: xs[:, c] = X^T * weights[:, c] (bf16 2x).
            xs = xspool.tile([P, N_CTRL, KS, P], BF16)
            for c in range(N_CTRL):
                nc.vector.tensor_mul(
                    xs[:, c],
                    xt,
                    wb[:, c, None, ts(t, P)].to_broadcast([P, KS, P]),
                )

            # Bias + all expert matmuls accumulate into one PSUM bank.
            out_ps = mm_psum.tile([P, D_OUT], F32)
            nc.tensor.matmul(
                out_ps,
                lhsT=wt_pad[:, ts(t, P)],
                rhs=b_pad,
                start=True,
                stop=False,
            )
            for c in range(N_CTRL):
                for k in range(KS):
                    nc.tensor.matmul(
                        out_ps,
                        lhsT=xs[:, c, k, :],
                        rhs=w_sb[:, c, k, :],
                        start=False,
                        stop=(c == N_CTRL - 1 and k == KS - 1),
                    )

            o_sb = opool.tile([P, D_OUT], F32)
            nc.scalar.copy(o_sb, out_ps)
            nc.sync.dma_start(o_d[ts(t, P), :], o_sb)


def _split_multi_waits(bir: dict) -> dict:
    """The walrus build in this container supports at most ONE sync-wait per
    instruction ("Too many sync wait commands" at codegen otherwise).  Tile's
    scheduler freely attaches several.  Split: keep the last wait on the
    instruction and hoist the others onto standalone same-engine
    EventSemaphore instructions inserted immediately before it — identical
    semantics (the engine blocks at the same program point)."""
    ctr = 0
    for func in bir["functions"]:
        for bb in func["blocks"]:
            new_insts = []
            for inst in bb["instructions"]:
                si = inst.get("sync_info")
                waits = si.get("on_wait") if si else None
                if waits and len(waits) > 1:
                    for w in waits[:-1]:
                        ctr += 1
                        new_insts.append(
                            {
                                "debug": inst.get("debug", 0),
                                "engine": inst["engine"],
                                "ins": [],
                                "outs": [],
                                "name": f"{inst['name']}-wsplit{ctr}",
                                "opcode": "EventSemaphore",
                                "sync_info": {"on_update": [], "on_wait": [w]},
                            }
                        )
                    si["on_wait"] = [waits[-1]]
                new_insts.append(inst)
            bb["instructions"] = new_insts
    return bir


_ORIG_TO_JSON_BYTES = bass.Bass.to_json_bytes


def _patched_to_json_bytes(self) -> bytes:
    bir = json.loads(_ORIG_TO_JSON_BYTES(self))
    _split_multi_waits(bir)
    return json.dumps(bir).encode()


_NC_CACHE = []


def _build() -> bass.Bass:
    if _NC_CACHE:
        return _NC_CACHE[0]
    nc = bass.Bass(
        "TRN2",
        target_bir_lowering=False,
        debug=False,
        enable_asserts=False,
        num_devices=N_CORES,
    )
    x_d = nc.dram_tensor("x_in", [B_SHARD, D_IN], F32, kind="ExternalInput").ap()
    wt_d = nc.dram_tensor("wt_in", [B_SHARD, N_CTRL], F32, kind="ExternalInput").ap()
    w_d = nc.dram_tensor("w_in", [N_CTRL, D_IN, D_OUT], F32, kind="ExternalInput").ap()
    b_d = nc.dram_tensor("b_in", [N_CTRL, D_OUT], F32, kind="ExternalInput").ap()
    o_d = nc.dram_tensor("out", [B_SHARD, D_OUT], F32, kind="ExternalOutput").ap()
    with tile.TileContext(nc) as tc:
        _body(nc, tc, x_d, wt_d, w_d, b_d, o_d)
    nc.to_json_bytes = types.MethodType(_patched_to_json_bytes, nc)
    _NC_CACHE.append(nc)
    return nc


def kernel(inputs, weights, w, b, _trace=False):
    nc = _build()
    inputs = np.ascontiguousarray(inputs, dtype=np.float32)
    weights = np.ascontiguousarray(weights, dtype=np.float32)
    w = np.ascontiguousarray(w, dtype=np.float32)
    b = np.ascontiguousarray(b, dtype=np.float32)

    in_maps = []
    for i in range(N_CORES):
        sl = slice(i * B_SHARD, (i + 1) * B_SHARD)
        in_maps.append(
            {
                "x_in": inputs[sl],
                "wt_in": weights[sl],
                "w_in": w,
                "b_in": b,
            }
        )
    res = run_bass_kernel_spmd(
        nc, in_maps, core_ids=list(range(N_CORES)), trace=_trace
    )
    out = np.concatenate([r["out"] for r in res.results], axis=0)
    if _trace:
        return out, res
    return out


# revision 10
# speedup vs baseline: 13568.3626x; 13568.3626x over previous
"""Mode-adaptive linear (MoE soft routing) Trainium2 kernel.

out[b, o] = sum_c weights[b, c] * (inputs[b, :] @ w[c])[o] + (weights @ bias)[b, o]

Strategy: data-parallel shard of the batch across 8 NeuronCores (1024 rows
each); w/bias replicated.  On each core the routing weights are folded into
the transposed input tiles (xs_c = weights[:, c] * X^T in bf16), so all 8
expert matmuls plus the bias matmul accumulate into a single PSUM bank per
128-row batch tile — equivalent to one K=4104 matmul per tile.
"""

import json
import types

import numpy as np

import concourse.bass as bass
import concourse.mybir as mybir
import concourse.tile as tile
from concourse.bass import ts
from concourse.bass_utils import run_bass_kernel_spmd
from concourse.masks import make_identity

N_CORES = 8
B, D_IN, D_OUT, N_CTRL = 8192, 512, 512, 8
B_SHARD = B // N_CORES          # 1024 rows per core
P = 128
N_TILES = B_SHARD // P          # 8 batch tiles per core
KS = D_IN // P                  # 4 K-chunks of 128
F32 = mybir.dt.float32
BF16 = mybir.dt.bfloat16


def _consts(nc: bass.Bass, const_pool):
    """One-time constants that consume engine registers to build (identity for
    PE transpose, the expert-selection matrix).  Negligible runtime."""
    identity = const_pool.tile([P, P], F32)
    make_identity(nc, identity)

    # Selection matrix: e_pad[p, c, m] = 1.0 iff p == c, so
    # matmul(lhsT=e_pad[:, c], rhs=wt_pad) broadcasts wt_pad row c to all
    # 128 output partitions.
    e_pad = const_pool.tile([P, N_CTRL, P], BF16)
    nc.gpsimd.memset(e_pad, 0.0)
    nc.gpsimd.affine_select(
        out=e_pad,
        in_=e_pad,
        compare_op=mybir.AluOpType.not_equal,
        fill=1.0,
        base=0,
        # predicate: p - c != 0 ? keep in_ (0) : fill (1)
        pattern=[[-1, N_CTRL], [0, P]],
        channel_multiplier=1,
    )
    return identity, e_pad


def _body(nc: bass.Bass, tc: tile.TileContext, x_d, wt_d, w_d, b_d, o_d,
          identity, e_pad):
    with (
        tc.tile_pool(name="const", bufs=1) as const_pool,
        tc.tile_pool(name="wstage", bufs=2) as wstage,
        tc.tile_pool(name="xpool", bufs=3) as xpool,
        tc.tile_pool(name="xtpool", bufs=3) as xtpool,
        tc.tile_pool(name="xspool", bufs=2) as xspool,
        tc.tile_pool(name="opool", bufs=3) as opool,
        tc.tile_pool(name="tr_ps", bufs=2, space="PSUM") as tr_psum,
        tc.tile_pool(name="mm_ps", bufs=2, space="PSUM") as mm_psum,
        tc.tile_pool(name="bc_ps", bufs=2, space="PSUM") as bc_psum,
    ):

        # Routing weights transposed to [8, 1024] via one small strided DMA,
        # then zero-padded to 128 partitions in bf16 (rows 8..127 = 0 so it
        # can serve as a K=128 matmul operand).
        wt_t_f32 = const_pool.tile([N_CTRL, B_SHARD], F32)
        wt_pad = const_pool.tile([P, B_SHARD], BF16)
        nc.gpsimd.memset(wt_pad, 0.0)
        WT_CHUNK = 256
        with nc.allow_non_contiguous_dma(
            reason="one-time 32KB routing-weight transpose load"
        ):
            for h in range(B_SHARD // WT_CHUNK):
                nc.sync.dma_start(
                    wt_t_f32[:, ts(h, WT_CHUNK)],
                    wt_d[ts(h, WT_CHUNK), :].rearrange("b c -> c b"),
                )
                nc.vector.tensor_copy(
                    wt_pad[0:N_CTRL, ts(h, WT_CHUNK)], wt_t_f32[:, ts(h, WT_CHUNK)]
                )

        # Bias, zero-padded the same way.
        b_f32 = const_pool.tile([N_CTRL, D_OUT], F32)
        nc.sync.dma_start(b_f32, b_d)
        b_pad = const_pool.tile([P, D_OUT], BF16)
        nc.gpsimd.memset(b_pad, 0.0)
        nc.vector.tensor_copy(b_pad[0:N_CTRL, :], b_f32)

        # Wb[p, c, b] = weights[b, c] for every partition p.
        wb = const_pool.tile([P, N_CTRL, B_SHARD], BF16)
        for c in range(N_CTRL):
            for h in range(B_SHARD // 512):
                bc_ps = bc_psum.tile([P, 512], F32)
                nc.tensor.matmul(
                    bc_ps,
                    lhsT=e_pad[:, c, :],
                    rhs=wt_pad[:, ts(h, 512)],
                    start=True,
                    stop=True,
                )
                nc.scalar.copy(wb[:, c, ts(h, 512)], bc_ps)

        # Expert weights: [128 (i%128), expert, i//128, o] in bf16.
        w_sb = const_pool.tile([P, N_CTRL, KS, D_OUT], BF16)
        for c in range(N_CTRL):
            w_f32 = wstage.tile([P, KS, D_OUT], F32)
            nc.sync.dma_start(w_f32, w_d[c].rearrange("(k p) o -> p k o", p=P))
            if c % 2 == 0:
                nc.vector.tensor_copy(w_sb[:, c], w_f32)
            else:
                nc.scalar.copy(w_sb[:, c], w_f32)

        for t in range(N_TILES):
            x_f32 = xpool.tile([P, D_IN], F32)
            nc.sync.dma_start(x_f32, x_d[ts(t, P), :])

            # X^T for this tile via PE transpose, cast to bf16 on ScalarE.
            tr_ps = tr_psum.tile([P, KS, P], F32)
            xt = xtpool.tile([P, KS, P], BF16)
            for k in range(KS):
                nc.tensor.transpose(tr_ps[:, k, :], x_f32[:, ts(k, P)], identity)
                nc.scalar.copy(xt[:, k, :], tr_ps[:, k, :])

            # Fold routing weights in: xs[:, c] = X^T * weights[:, c] (bf16 2x).
            xs = xspool.tile([P, N_CTRL, KS, P], BF16)
            for c in range(N_CTRL):
                nc.vector.tensor_mul(
                    xs[:, c],
                    xt,
                    wb[:, c, None, ts(t, P)].to_broadcast([P, KS, P]),
                )

            # Bias + all expert matmuls accumulate into one PSUM bank.
            out_ps = mm_psum.tile([P, D_OUT], F32)
            nc.tensor.matmul(
                out_ps,
                lhsT=wt_pad[:, ts(t, P)],
                rhs=b_pad,
                start=True,
                stop=False,
            )
            for c in range(N_CTRL):
                for k in range(KS):
                    nc.tensor.matmul(
                        out_ps,
                        lhsT=xs[:, c, k, :],
                        rhs=w_sb[:, c, k, :],
                        start=False,
                        stop=(c == N_CTRL - 1 and k == KS - 1),
                    )

            o_sb = opool.tile([P, D_OUT], F32)
            nc.scalar.copy(o_sb, out_ps)
            nc.sync.dma_start(o_d[ts(t, P), :], o_sb)


def _split_multi_waits(bir: dict) -> dict:
    """The walrus build in this container supports at most ONE sync-wait per
    instruction ("Too many sync wait commands" at codegen otherwise).  Tile's
    scheduler freely attaches several.  Split: keep the last wait on the
    instruction and hoist the others onto standalone same-engine
    EventSemaphore instructions inserted immediately before it — identical
    semantics (the engine blocks at the same program point)."""
    ctr = 0
    for func in bir["functions"]:
        for bb in func["blocks"]:
            new_insts = []
            for inst in bb["instructions"]:
                si = inst.get("sync_info")
                waits = si.get("on_wait") if si else None
                if waits and len(waits) > 1:
                    for w in waits[:-1]:
                        ctr += 1
                        new_insts.append(
                            {
                                "debug": inst.get("debug", 0),
                                "engine": inst["engine"],
                                "ins": [],
                                "outs": [],
                                "name": f"{inst['name']}-wsplit{ctr}",
                                "opcode": "EventSemaphore",
                                "sync_info": {"on_update": [], "on_wait": [w]},
                            }
                        )
                    si["on_wait"] = [waits[-1]]
                new_insts.append(inst)
            bb["instructions"] = new_insts
    return bir


_ORIG_TO_JSON_BYTES = bass.Bass.to_json_bytes


def _patched_to_json_bytes(self) -> bytes:
    bir = json.loads(_ORIG_TO_JSON_BYTES(self))
    _split_multi_waits(bir)
    return json.dumps(bir).encode()


_NC_CACHE = []


def _build() -> bass.Bass:
    if _NC_CACHE:
        return _NC_CACHE[0]
    nc = bass.Bass(
        "TRN2",
        target_bir_lowering=False,
        debug=False,
        enable_asserts=False,
        num_devices=N_CORES,
    )
    x_d = nc.dram_tensor("x_in", [B_SHARD, D_IN], F32, kind="ExternalInput").ap()
    wt_d = nc.dram_tensor("wt_in", [B_SHARD, N_CTRL], F32, kind="ExternalInput").ap()
    w_d = nc.dram_tensor("w_in", [N_CTRL, D_IN, D_OUT], F32, kind="ExternalInput").ap()
    b_d = nc.dram_tensor("b_in", [N_CTRL, D_OUT], F32, kind="ExternalInput").ap()
    o_d = nc.dram_tensor("out", [B_SHARD, D_OUT], F32, kind="ExternalOutput").ap()
    with tile.TileContext(nc) as tc:
        with tc.tile_pool(name="global_const", bufs=1) as gconst:
            identity, e_pad = _consts(nc, gconst)
            _body(nc, tc, x_d, wt_d, w_d, b_d, o_d, identity, e_pad)
    nc.to_json_bytes = types.MethodType(_patched_to_json_bytes, nc)
    _NC_CACHE.append(nc)
    return nc


def kernel(inputs, weights, w, b, _trace=False):
    nc = _build()
    inputs = np.ascontiguousarray(inputs, dtype=np.float32)
    weights = np.ascontiguousarray(weights, dtype=np.float32)
    w = np.ascontiguousarray(w, dtype=np.float32)
    b = np.ascontiguousarray(b, dtype=np.float32)

    in_maps = []
    for i in range(N_CORES):
        sl = slice(i * B_SHARD, (i + 1) * B_SHARD)
        in_maps.append(
            {
                "x_in": inputs[sl],
                "wt_in": weights[sl],
                "w_in": w,
                "b_in": b,
            }
        )
    res = run_bass_kernel_spmd(
        nc, in_maps, core_ids=list(range(N_CORES)), trace=_trace
    )
    out = np.concatenate([r["out"] for r in res.results], axis=0)
    if _trace:
        return out, res
    return out


# revision 19
# speedup vs baseline: 15378.7533x; 1.1334x over previous
"""Mode-adaptive linear (MoE soft routing) Trainium2 kernel.

out[b, o] = sum_c weights[b, c] * (inputs[b, :] @ w[c])[o] + (weights @ bias)[b, o]

Strategy: data-parallel shard of the batch across 8 NeuronCores (1024 rows
each); w/bias replicated.  On each core the routing weights are folded into
the transposed input tiles (xs_c = weights[:, c] * X^T in bf16), so all 8
expert matmuls plus the bias matmul accumulate into a single PSUM bank per
128-row batch tile — equivalent to one K=4104 matmul per tile.
"""

import json
import types

import numpy as np

import concourse.bass as bass
import concourse.mybir as mybir
import concourse.tile as tile
from concourse.bass import ts
from concourse.bass_utils import run_bass_kernel_spmd
from concourse.masks import make_identity

N_CORES = 8
B, D_IN, D_OUT, N_CTRL = 8192, 512, 512, 8
B_SHARD = B // N_CORES          # 1024 rows per core
P = 128
N_TILES = B_SHARD // P          # 8 batch tiles per core
KS = D_IN // P                  # 4 K-chunks of 128
F32 = mybir.dt.float32
BF16 = mybir.dt.bfloat16


def _consts(nc: bass.Bass, const_pool):
    """One-time constants, embedded in the NEFF and DMA'd to SBUF (no engine
    work): identity for PE transpose, and the expert-selection matrix
    e_pad[p, c, m] = 1 iff p == c, so matmul(lhsT=e_pad[:, c], rhs=wt_pad)
    broadcasts wt_pad row c to all 128 output partitions."""
    import ml_dtypes

    identity_d = nc.inline_tensor(np.eye(P, dtype=np.float32), name="identity_const")
    identity = const_pool.tile([P, P], F32)
    nc.sync.dma_start(identity, identity_d.ap())

    e_np = np.zeros((P, N_CTRL, P), dtype=ml_dtypes.bfloat16)
    for c in range(N_CTRL):
        e_np[c, c, :] = 1.0
    e_d = nc.inline_tensor(e_np, name="e_pad_const")
    e_pad = const_pool.tile([P, N_CTRL, P], BF16)
    nc.sync.dma_start(e_pad, e_d.ap())
    return identity, e_pad


def _body(nc: bass.Bass, tc: tile.TileContext, x_d, wt_d, w_d, b_d, o_d,
          identity, e_pad):
    with (
        tc.tile_pool(name="const", bufs=1) as const_pool,
        tc.tile_pool(name="wstage", bufs=2) as wstage,
        tc.tile_pool(name="xpool", bufs=4) as xpool,
        tc.tile_pool(name="xtpool", bufs=N_TILES) as xtpool,
        tc.tile_pool(name="xspool", bufs=3) as xspool,
        tc.tile_pool(name="opool", bufs=3) as opool,
        tc.tile_pool(name="tr_ps", bufs=3, space="PSUM") as tr_psum,
        tc.tile_pool(name="mm_ps", bufs=3, space="PSUM") as mm_psum,
    ):
        bc_psum = mm_psum  # share banks: bc only used during setup

        # --- Phase 1: small loads + everything not needing the 8MB w ---

        # First two x tiles right away so PE transposes can start ASAP.
        x_f32s = []
        for t in range(2):
            x_f32 = xpool.tile([P, D_IN], F32, tag="x_f32")
            nc.sync.dma_start(x_f32, x_d[ts(t, P), :])
            x_f32s.append(x_f32)

        # Routing weights transposed to [8, 1024] via small strided DMAs,
        # then zero-padded to 128 partitions in bf16 (rows 8..127 = 0 so it
        # can serve as a K=128 matmul operand).
        wt_t_f32 = const_pool.tile([N_CTRL, B_SHARD], F32)
        wt_pad = const_pool.tile([P, B_SHARD], BF16)
        nc.gpsimd.memset(wt_pad, 0.0)
        WT_CHUNK = 256
        with nc.allow_non_contiguous_dma(
            reason="one-time 32KB routing-weight transpose load"
        ):
            for h in range(B_SHARD // WT_CHUNK):
                nc.sync.dma_start(
                    wt_t_f32[:, ts(h, WT_CHUNK)],
                    wt_d[ts(h, WT_CHUNK), :].rearrange("b c -> c b"),
                )
                nc.vector.tensor_copy(
                    wt_pad[0:N_CTRL, ts(h, WT_CHUNK)], wt_t_f32[:, ts(h, WT_CHUNK)]
                )

        # Bias, zero-padded the same way.
        b_f32 = const_pool.tile([N_CTRL, D_OUT], F32)
        nc.sync.dma_start(b_f32, b_d)
        b_pad = const_pool.tile([P, D_OUT], BF16)
        nc.gpsimd.memset(b_pad, 0.0)
        nc.vector.tensor_copy(b_pad[0:N_CTRL, :], b_f32)

        # X^T per tile via PE transpose (cast to bf16 on ScalarE), with the
        # Wb broadcast matmuls interleaved after the first two tiles so PE
        # has work while later x tiles stream in.
        # Wb[p, c, b] = weights[b, c] for every partition p.
        wb = const_pool.tile([P, N_CTRL, B_SHARD], BF16)
        xts = []

        def transpose_tile(t):
            if t < 2:
                x_f32 = x_f32s[t]
            else:
                x_f32 = xpool.tile([P, D_IN], F32, tag="x_f32")
                nc.sync.dma_start(x_f32, x_d[ts(t, P), :])
            tr_ps = tr_psum.tile([P, KS, P], F32)
            xt = xtpool.tile([P, KS, P], BF16)
            for k in range(KS):
                nc.tensor.transpose(tr_ps[:, k, :], x_f32[:, ts(k, P)], identity)
                nc.scalar.copy(xt[:, k, :], tr_ps[:, k, :])
            xts.append(xt)

        for t in range(2):
            transpose_tile(t)
        for c in range(N_CTRL):
            for h in range(B_SHARD // 512):
                bc_ps = bc_psum.tile([P, 512], F32, tag="bc", bufs=2)
                nc.tensor.matmul(
                    bc_ps,
                    lhsT=e_pad[:, c, :],
                    rhs=wt_pad[:, ts(h, 512)],
                    start=True,
                    stop=True,
                )
                nc.scalar.copy(wb[:, c, ts(h, 512)], bc_ps)
        for t in range(2, N_TILES):
            transpose_tile(t)

        # --- Phase 2: bulk expert-weight load + cast, one DMA per (expert,
        # K-chunk) so each matmul's operand arrives & casts independently ---
        # [128 (i%128), expert, i//128, o] in bf16.
        w_sb = const_pool.tile([P, N_CTRL, KS, D_OUT], BF16)
        for c in range(N_CTRL):
            for k in range(KS):
                w_f32 = wstage.tile([P, D_OUT], F32, tag="w_f32", bufs=6)
                nc.sync.dma_start(
                    w_f32, w_d[c, ts(k, P), :]
                )
                if (c * KS + k) % 2 == 0:
                    nc.vector.tensor_copy(w_sb[:, c, k], w_f32)
                else:
                    nc.scalar.copy(w_sb[:, c, k], w_f32)

        # --- Phase 3: scale + matmul-accumulate, experts in 2 groups of 4 so
        # PE only ever waits on the first half of the w load; group A partial
        # sums park in SBUF, group B adds them back before the store. ---
        GROUPS = 2
        CPG = N_CTRL // GROUPS
        accs = []
        for g in range(GROUPS):
            for t in range(N_TILES):
                # Fold routing weights in: xs[:, c] = X^T * weights (bf16 2x).
                xs = xspool.tile([P, CPG, KS, P], BF16)
                for ci in range(CPG):
                    c = g * CPG + ci
                    nc.vector.tensor_mul(
                        xs[:, ci],
                        xts[t],
                        wb[:, c, None, ts(t, P)].to_broadcast([P, KS, P]),
                    )

                out_ps = mm_psum.tile([P, D_OUT], F32, tag="acc")
                if g == 0:
                    # Bias rides in group A's accumulation.
                    nc.tensor.matmul(
                        out_ps,
                        lhsT=wt_pad[:, ts(t, P)],
                        rhs=b_pad,
                        start=True,
                        stop=False,
                    )
                for ci in range(CPG):
                    c = g * CPG + ci
                    for k in range(KS):
                        nc.tensor.matmul(
                            out_ps,
                            lhsT=xs[:, ci, k, :],
                            rhs=w_sb[:, c, k, :],
                            start=(g != 0 and ci == 0 and k == 0),
                            stop=(ci == CPG - 1 and k == KS - 1),
                        )

                if g == 0:
                    acc = opool.tile([P, D_OUT], F32, tag="acc_sb", bufs=N_TILES)
                    nc.scalar.copy(acc, out_ps)
                    accs.append(acc)
                else:
                    o_sb = opool.tile([P, D_OUT], F32, tag="o_sb")
                    nc.vector.tensor_add(o_sb, out_ps, accs[t])
                    nc.sync.dma_start(o_d[ts(t, P), :], o_sb)


def _split_multi_waits(bir: dict) -> dict:
    """The walrus build in this container supports at most ONE sync-wait per
    instruction ("Too many sync wait commands" at codegen otherwise).  Tile's
    scheduler freely attaches several.  Split: keep the last wait on the
    instruction and hoist the others onto standalone same-engine
    EventSemaphore instructions inserted immediately before it — identical
    semantics (the engine blocks at the same program point)."""
    ctr = 0
    for func in bir["functions"]:
        for bb in func["blocks"]:
            new_insts = []
            for inst in bb["instructions"]:
                si = inst.get("sync_info")
                waits = si.get("on_wait") if si else None
                if waits and len(waits) > 1:
                    for w in waits[:-1]:
                        ctr += 1
                        new_insts.append(
                            {
                                "debug": inst.get("debug", 0),
                                "engine": inst["engine"],
                                "ins": [],
                                "outs": [],
                                "name": f"{inst['name']}-wsplit{ctr}",
                                "opcode": "EventSemaphore",
                                "sync_info": {"on_update": [], "on_wait": [w]},
                            }
                        )
                    si["on_wait"] = [waits[-1]]
                new_insts.append(inst)
            bb["instructions"] = new_insts
    return bir


_ORIG_TO_JSON_BYTES = bass.Bass.to_json_bytes


def _patched_to_json_bytes(self) -> bytes:
    bir = json.loads(_ORIG_TO_JSON_BYTES(self))
    _split_multi_waits(bir)
    return json.dumps(bir).encode()


_NC_CACHE = []


def _build() -> bass.Bass:
    if _NC_CACHE:
        return _NC_CACHE[0]
    nc = bass.Bass(
        "TRN2",
        target_bir_lowering=False,
        debug=False,
        enable_asserts=False,
        num_devices=N_CORES,
    )
    x_d = nc.dram_tensor("x_in", [B_SHARD, D_IN], F32, kind="ExternalInput").ap()
    wt_d = nc.dram_tensor("wt_in", [B_SHARD, N_CTRL], F32, kind="ExternalInput").ap()
    w_d = nc.dram_tensor("w_in", [N_CTRL, D_IN, D_OUT], F32, kind="ExternalInput").ap()
    b_d = nc.dram_tensor("b_in", [N_CTRL, D_OUT], F32, kind="ExternalInput").ap()
    o_d = nc.dram_tensor("out", [B_SHARD, D_OUT], F32, kind="ExternalOutput").ap()
    with tile.TileContext(nc) as tc:
        with tc.tile_pool(name="global_const", bufs=1) as gconst:
            identity, e_pad = _consts(nc, gconst)
            _body(nc, tc, x_d, wt_d, w_d, b_d, o_d, identity, e_pad)
    nc.to_json_bytes = types.MethodType(_patched_to_json_bytes, nc)
    _NC_CACHE.append(nc)
    return nc


def kernel(inputs, weights, w, b, _trace=False):
    nc = _build()
    inputs = np.ascontiguousarray(inputs, dtype=np.float32)
    weights = np.ascontiguousarray(weights, dtype=np.float32)
    w = np.ascontiguousarray(w, dtype=np.float32)
    b = np.ascontiguousarray(b, dtype=np.float32)

    in_maps = []
    for i in range(N_CORES):
        sl = slice(i * B_SHARD, (i + 1) * B_SHARD)
        in_maps.append(
            {
                "x_in": inputs[sl],
                "wt_in": weights[sl],
                "w_in": w,
                "b_in": b,
            }
        )
    res = run_bass_kernel_spmd(
        nc, in_maps, core_ids=list(range(N_CORES)), trace=_trace
    )
    out = np.concatenate([r["out"] for r in res.results], axis=0)
    if _trace:
        return out, res
    return out


# revision 24
# speedup vs baseline: 17298.2468x; 1.1248x over previous
"""Mode-adaptive linear (MoE soft routing) Trainium2 kernel.

out[b, o] = sum_c weights[b, c] * (inputs[b, :] @ w[c])[o] + (weights @ bias)[b, o]

Strategy: data-parallel shard of the batch across 8 NeuronCores (1024 rows
each); w/bias replicated.  On each core the routing weights are folded into
the transposed input tiles (xs_c = weights[:, c] * X^T in bf16), so all 8
expert matmuls plus the bias matmul accumulate into a single PSUM bank per
128-row batch tile — equivalent to one K=4104 matmul per tile.
"""

import json
import types

import numpy as np

import concourse.bass as bass
import concourse.mybir as mybir
import concourse.tile as tile
from concourse.bass import ts
from concourse.bass_utils import run_bass_kernel_spmd
from concourse.masks import make_identity

N_CORES = 8
B, D_IN, D_OUT, N_CTRL = 8192, 512, 512, 8
B_SHARD = B // N_CORES          # 1024 rows per core
P = 128
N_TILES = B_SHARD // P          # 8 batch tiles per core
KS = D_IN // P                  # 4 K-chunks of 128
F32 = mybir.dt.float32
BF16 = mybir.dt.bfloat16


def _consts(nc: bass.Bass, const_pool):
    """One-time constants, embedded in the NEFF and DMA'd to SBUF (no engine
    work): identity for PE transpose, and the expert-selection matrix
    e_pad[p, c, m] = 1 iff p == c, so matmul(lhsT=e_pad[:, c], rhs=wt_pad)
    broadcasts wt_pad row c to all 128 output partitions."""
    import ml_dtypes

    identity_d = nc.inline_tensor(np.eye(P, dtype=np.float32), name="identity_const")
    identity = const_pool.tile([P, P], F32)
    nc.sync.dma_start(identity, identity_d.ap())

    e_np = np.zeros((P, N_CTRL, P), dtype=ml_dtypes.bfloat16)
    for c in range(N_CTRL):
        e_np[c, c, :] = 1.0
    e_d = nc.inline_tensor(e_np, name="e_pad_const")
    e_pad = const_pool.tile([P, N_CTRL, P], BF16)
    nc.sync.dma_start(e_pad, e_d.ap())
    return identity, e_pad


def _body(nc: bass.Bass, tc: tile.TileContext, x_d, wt_d, w_d, b_d, o_d,
          identity, e_pad):
    with (
        tc.tile_pool(name="const", bufs=1) as const_pool,
        tc.tile_pool(name="wstage", bufs=2) as wstage,
        tc.tile_pool(name="xpool", bufs=4) as xpool,
        tc.tile_pool(name="xtpool", bufs=N_TILES) as xtpool,
        tc.tile_pool(name="xspool", bufs=3) as xspool,
        tc.tile_pool(name="opool", bufs=3) as opool,
        tc.tile_pool(name="tr_ps", bufs=3, space="PSUM") as tr_psum,
        tc.tile_pool(name="mm_ps", bufs=3, space="PSUM") as mm_psum,
    ):
        bc_psum = mm_psum  # share banks: bc only used during setup

        # --- Phase 1: small loads + everything not needing the 8MB w ---

        # First two x tiles right away so PE transposes can start ASAP.
        x_f32s = []
        for t in range(2):
            x_f32 = xpool.tile([P, D_IN], F32, tag="x_f32")
            nc.sync.dma_start(x_f32, x_d[ts(t, P), :])
            x_f32s.append(x_f32)

        # Routing weights transposed to [8, 1024] via small strided DMAs,
        # then zero-padded to 128 partitions in bf16 (rows 8..127 = 0 so it
        # can serve as a K=128 matmul operand).
        wt_t_f32 = const_pool.tile([N_CTRL, B_SHARD], F32)
        wt_pad = const_pool.tile([P, B_SHARD], BF16)
        nc.gpsimd.memset(wt_pad, 0.0)
        WT_CHUNK = 256
        with nc.allow_non_contiguous_dma(
            reason="one-time 32KB routing-weight transpose load"
        ):
            for h in range(B_SHARD // WT_CHUNK):
                nc.sync.dma_start(
                    wt_t_f32[:, ts(h, WT_CHUNK)],
                    wt_d[ts(h, WT_CHUNK), :].rearrange("b c -> c b"),
                )
                nc.vector.tensor_copy(
                    wt_pad[0:N_CTRL, ts(h, WT_CHUNK)], wt_t_f32[:, ts(h, WT_CHUNK)]
                )

        # Bias, zero-padded the same way.
        b_f32 = const_pool.tile([N_CTRL, D_OUT], F32)
        nc.sync.dma_start(b_f32, b_d)
        b_pad = const_pool.tile([P, D_OUT], BF16)
        nc.gpsimd.memset(b_pad, 0.0)
        nc.vector.tensor_copy(b_pad[0:N_CTRL, :], b_f32)

        # X^T per tile via PE transpose (cast to bf16 on ScalarE), with the
        # Wb broadcast matmuls interleaved after the first two tiles so PE
        # has work while later x tiles stream in.
        # Wb[p, c, b] = weights[b, c] for every partition p.
        wb = const_pool.tile([P, N_CTRL, B_SHARD], BF16)
        xts = []

        def transpose_tile(t):
            if t < 2:
                x_f32 = x_f32s[t]
            else:
                x_f32 = xpool.tile([P, D_IN], F32, tag="x_f32")
                nc.sync.dma_start(x_f32, x_d[ts(t, P), :])
            tr_ps = tr_psum.tile([P, KS, P], F32)
            xt = xtpool.tile([P, KS, P], BF16)
            for k in range(KS):
                nc.tensor.transpose(tr_ps[:, k, :], x_f32[:, ts(k, P)], identity)
            nc.scalar.copy(xt, tr_ps)
            xts.append(xt)

        for t in range(2):
            transpose_tile(t)
        for c in range(N_CTRL):
            for h in range(B_SHARD // 512):
                bc_ps = bc_psum.tile([P, 512], F32, tag="bc", bufs=2)
                nc.tensor.matmul(
                    bc_ps,
                    lhsT=e_pad[:, c, :],
                    rhs=wt_pad[:, ts(h, 512)],
                    start=True,
                    stop=True,
                )
                nc.scalar.copy(wb[:, c, ts(h, 512)], bc_ps)

        # --- Phase 2: bulk expert-weight load + cast, one DMA per (expert,
        # K-chunk).  The first expert group's chunks are issued before the
        # remaining x tiles so the matmul stream is never starved. ---
        # [128 (i%128), expert, i//128, o] in bf16.
        w_sb = const_pool.tile([P, N_CTRL, KS, D_OUT], BF16)

        def load_w(c):
            for k in range(KS):
                w_f32 = wstage.tile([P, D_OUT], F32, tag="w_f32", bufs=6)
                nc.sync.dma_start(w_f32, w_d[c, ts(k, P), :])
                if (c * KS + k) % 2 == 0:
                    nc.vector.tensor_copy(w_sb[:, c, k], w_f32)
                else:
                    nc.scalar.copy(w_sb[:, c, k], w_f32)

        load_w(0)
        load_w(1)
        for t in range(2, N_TILES):
            transpose_tile(t)
            if t - 2 < 6:
                load_w(t)  # experts 2..7 behind tiles 2..7
        # (experts 2..7 all covered by the loop above since N_TILES-2 == 6)

        # --- Phase 3: scale + matmul-accumulate, experts in 4 groups of 2 so
        # PE only ever waits on a quarter of the w load; partial sums chain
        # through an SBUF accumulator per tile. ---
        GROUPS = 2
        CPG = N_CTRL // GROUPS
        accs = [None] * N_TILES
        for g in range(GROUPS):
            for t in range(N_TILES):
                # Fold routing weights in: xs[:, ci] = X^T * weights[:, c]
                # — one DVE op for the whole expert group (bf16 2x).
                xs = xspool.tile([P, CPG, KS, P], BF16)
                nc.vector.tensor_mul(
                    xs,
                    xts[t][:, None, :, :].to_broadcast([P, CPG, KS, P]),
                    wb[:, ts(g, CPG), None, ts(t, P)].to_broadcast(
                        [P, CPG, KS, P]
                    ),
                )

                out_ps = mm_psum.tile([P, D_OUT], F32, tag="acc")
                if g == 0:
                    # Bias rides in the first group's accumulation.
                    nc.tensor.matmul(
                        out_ps,
                        lhsT=wt_pad[:, ts(t, P)],
                        rhs=b_pad,
                        start=True,
                        stop=False,
                    )
                for ci in range(CPG):
                    c = g * CPG + ci
                    for k in range(KS):
                        nc.tensor.matmul(
                            out_ps,
                            lhsT=xs[:, ci, k, :],
                            rhs=w_sb[:, c, k, :],
                            start=(g != 0 and ci == 0 and k == 0),
                            stop=(ci == CPG - 1 and k == KS - 1),
                        )

                if g == 0:
                    acc = opool.tile([P, D_OUT], F32, tag="acc_sb", bufs=N_TILES)
                    nc.scalar.copy(acc, out_ps)
                    accs[t] = acc
                elif g < GROUPS - 1:
                    # acc += psum (in-place on DVE)
                    nc.vector.tensor_add(accs[t], out_ps, accs[t])
                else:
                    o_sb = opool.tile([P, D_OUT], F32, tag="o_sb")
                    nc.vector.tensor_add(o_sb, out_ps, accs[t])
                    nc.sync.dma_start(o_d[ts(t, P), :], o_sb)


def _split_multi_waits(bir: dict) -> dict:
    """The walrus build in this container supports at most ONE sync-wait per
    instruction ("Too many sync wait commands" at codegen otherwise).  Tile's
    scheduler freely attaches several.  Split: keep the last wait on the
    instruction and hoist the others onto standalone same-engine
    EventSemaphore instructions inserted immediately before it — identical
    semantics (the engine blocks at the same program point)."""
    ctr = 0
    for func in bir["functions"]:
        for bb in func["blocks"]:
            new_insts = []
            for inst in bb["instructions"]:
                si = inst.get("sync_info")
                waits = si.get("on_wait") if si else None
                if waits and len(waits) > 1:
                    for w in waits[:-1]:
                        ctr += 1
                        new_insts.append(
                            {
                                "debug": inst.get("debug", 0),
                                "engine": inst["engine"],
                                "ins": [],
                                "outs": [],
                                "name": f"{inst['name']}-wsplit{ctr}",
                                "opcode": "EventSemaphore",
                                "sync_info": {"on_update": [], "on_wait": [w]},
                            }
                        )
                    si["on_wait"] = [waits[-1]]
                new_insts.append(inst)
            bb["instructions"] = new_insts
    return bir


_ORIG_TO_JSON_BYTES = bass.Bass.to_json_bytes


def _patched_to_json_bytes(self) -> bytes:
    bir = json.loads(_ORIG_TO_JSON_BYTES(self))
    _split_multi_waits(bir)
    return json.dumps(bir).encode()


_NC_CACHE = []


def _build() -> bass.Bass:
    if _NC_CACHE:
        return _NC_CACHE[0]
    nc = bass.Bass(
        "TRN2",
        target_bir_lowering=False,
        debug=False,
        enable_asserts=False,
        num_devices=N_CORES,
    )
    x_d = nc.dram_tensor("x_in", [B_SHARD, D_IN], F32, kind="ExternalInput").ap()
    wt_d = nc.dram_tensor("wt_in", [B_SHARD, N_CTRL], F32, kind="ExternalInput").ap()
    w_d = nc.dram_tensor("w_in", [N_CTRL, D_IN, D_OUT], F32, kind="ExternalInput").ap()
    b_d = nc.dram_tensor("b_in", [N_CTRL, D_OUT], F32, kind="ExternalInput").ap()
    o_d = nc.dram_tensor("out", [B_SHARD, D_OUT], F32, kind="ExternalOutput").ap()
    with tile.TileContext(nc) as tc:
        with tc.tile_pool(name="global_const", bufs=1) as gconst:
            identity, e_pad = _consts(nc, gconst)
            _body(nc, tc, x_d, wt_d, w_d, b_d, o_d, identity, e_pad)
    nc.to_json_bytes = types.MethodType(_patched_to_json_bytes, nc)
    _NC_CACHE.append(nc)
    return nc


def kernel(inputs, weights, w, b, _trace=False):
    nc = _build()
    inputs = np.ascontiguousarray(inputs, dtype=np.float32)
    weights = np.ascontiguousarray(weights, dtype=np.float32)
    w = np.ascontiguousarray(w, dtype=np.float32)
    b = np.ascontiguousarray(b, dtype=np.float32)

    in_maps = []
    for i in range(N_CORES):
        sl = slice(i * B_SHARD, (i + 1) * B_SHARD)
        in_maps.append(
            {
                "x_in": inputs[sl],
                "wt_in": weights[sl],
                "w_in": w,
                "b_in": b,
            }
        )
    res = run_bass_kernel_spmd(
        nc, in_maps, core_ids=list(range(N_CORES)), trace=_trace
    )
    out = np.concatenate([r["out"] for r in res.results], axis=0)
    if _trace:
        return out, res
    return out


# revision 26
# speedup vs baseline: 20677.7930x; 1.1954x over previous
"""Mode-adaptive linear (MoE soft routing) Trainium2 kernel.

out[b, o] = sum_c weights[b, c] * (inputs[b, :] @ w[c])[o] + (weights @ bias)[b, o]

Strategy: data-parallel shard of the batch across 8 NeuronCores (1024 rows
each); w/bias replicated.  On each core the routing weights are folded into
the transposed input tiles (xs_c = weights[:, c] * X^T in bf16), so all 8
expert matmuls plus the bias matmul accumulate into a single PSUM bank per
128-row batch tile — equivalent to one K=4104 matmul per tile.
"""

import json
import types

import numpy as np

import concourse.bass as bass
import concourse.mybir as mybir
import concourse.tile as tile
from concourse.bass import ts
from concourse.bass_utils import run_bass_kernel_spmd
from concourse.masks import make_identity

N_CORES = 8
B, D_IN, D_OUT, N_CTRL = 8192, 512, 512, 8
B_SHARD = B // N_CORES          # 1024 rows per core
P = 128
N_TILES = B_SHARD // P          # 8 batch tiles per core
KS = D_IN // P                  # 4 K-chunks of 128
F32 = mybir.dt.float32
BF16 = mybir.dt.bfloat16


def _consts(nc: bass.Bass, const_pool):
    """One-time constants, embedded in the NEFF and DMA'd to SBUF (no engine
    work): identity for PE transpose, and the expert-selection matrix
    e_pad[p, c, m] = 1 iff p == c, so matmul(lhsT=e_pad[:, c], rhs=wt_pad)
    broadcasts wt_pad row c to all 128 output partitions."""
    import ml_dtypes

    identity_d = nc.inline_tensor(np.eye(P, dtype=np.float32), name="identity_const")
    identity = const_pool.tile([P, P], F32)
    nc.sync.dma_start(identity, identity_d.ap())

    e_np = np.zeros((P, N_CTRL, P), dtype=ml_dtypes.bfloat16)
    for c in range(N_CTRL):
        e_np[c, c, :] = 1.0
    e_d = nc.inline_tensor(e_np, name="e_pad_const")
    e_pad = const_pool.tile([P, N_CTRL, P], BF16)
    nc.sync.dma_start(e_pad, e_d.ap())
    return identity, e_pad


def _body(nc: bass.Bass, tc: tile.TileContext, x_d, wt_d, w_d, b_d, o_d,
          identity, e_pad):
    with (
        tc.tile_pool(name="const", bufs=1) as const_pool,
        tc.tile_pool(name="wstage", bufs=2) as wstage,
        tc.tile_pool(name="xpool", bufs=4) as xpool,
        tc.tile_pool(name="xtpool", bufs=N_TILES) as xtpool,
        tc.tile_pool(name="xspool", bufs=3) as xspool,
        tc.tile_pool(name="opool", bufs=3) as opool,
        tc.tile_pool(name="tr_ps", bufs=3, space="PSUM") as tr_psum,
        tc.tile_pool(name="mm_ps", bufs=3, space="PSUM") as mm_psum,
    ):
        bc_psum = mm_psum  # share banks: bc only used during setup

        # --- Phase 1: small loads + everything not needing the 8MB w ---

        # First two x tiles right away so PE transposes can start ASAP.
        x_f32s = []
        for t in range(2):
            x_f32 = xpool.tile([P, D_IN], F32, tag="x_f32")
            nc.sync.dma_start(x_f32, x_d[ts(t, P), :])
            x_f32s.append(x_f32)

        # Routing weights: load naturally as [128, tile, 8] (32B runs), then
        # one PE transpose per tile gives wt^T [8, 128] chunks, padded to 128
        # partitions in bf16 (rows 8..127 = 0 so wt_pad can serve as a K=128
        # matmul operand).
        wt_nat = const_pool.tile([P, N_TILES, N_CTRL], F32)
        nc.sync.dma_start(wt_nat, wt_d.rearrange("(t p) c -> p t c", p=P))
        wt_pad = const_pool.tile([P, B_SHARD], BF16)
        nc.gpsimd.memset(wt_pad, 0.0)
        for t in range(N_TILES):
            wtt_ps = mm_psum.tile([N_CTRL, P], F32, tag="bc", bufs=2)
            nc.tensor.transpose(wtt_ps, wt_nat[:, t, :], identity)
            nc.scalar.copy(wt_pad[0:N_CTRL, ts(t, P)], wtt_ps)

        # Bias, zero-padded the same way.
        b_f32 = const_pool.tile([N_CTRL, D_OUT], F32)
        nc.sync.dma_start(b_f32, b_d)
        b_pad = const_pool.tile([P, D_OUT], BF16)
        nc.gpsimd.memset(b_pad, 0.0)
        nc.vector.tensor_copy(b_pad[0:N_CTRL, :], b_f32)

        # X^T per tile via PE transpose (cast to bf16 on ScalarE), with the
        # Wb broadcast matmuls interleaved after the first two tiles so PE
        # has work while later x tiles stream in.
        # Wb[p, c, b] = weights[b, c] for every partition p.
        wb = const_pool.tile([P, N_CTRL, B_SHARD], BF16)
        xts = []

        def transpose_tile(t):
            if t < 2:
                x_f32 = x_f32s[t]
            else:
                x_f32 = xpool.tile([P, D_IN], F32, tag="x_f32")
                nc.sync.dma_start(x_f32, x_d[ts(t, P), :])
            tr_ps = tr_psum.tile([P, KS, P], F32)
            xt = xtpool.tile([P, KS, P], BF16)
            for k in range(KS):
                nc.tensor.transpose(tr_ps[:, k, :], x_f32[:, ts(k, P)], identity)
            nc.scalar.copy(xt, tr_ps)
            xts.append(xt)

        for t in range(2):
            transpose_tile(t)
        for c in range(N_CTRL):
            for h in range(B_SHARD // 512):
                bc_ps = bc_psum.tile([P, 512], F32, tag="bc", bufs=2)
                nc.tensor.matmul(
                    bc_ps,
                    lhsT=e_pad[:, c, :],
                    rhs=wt_pad[:, ts(h, 512)],
                    start=True,
                    stop=True,
                )
                nc.scalar.copy(wb[:, c, ts(h, 512)], bc_ps)

        # --- Phase 2: bulk expert-weight load + cast, one DMA per (expert,
        # K-chunk).  The first expert group's chunks are issued before the
        # remaining x tiles so the matmul stream is never starved. ---
        # [128 (i%128), expert, i//128, o] in bf16.
        w_sb = const_pool.tile([P, N_CTRL, KS, D_OUT], BF16)

        def load_w(c):
            for k in range(KS):
                w_f32 = wstage.tile([P, D_OUT], F32, tag="w_f32", bufs=6)
                nc.sync.dma_start(w_f32, w_d[c, ts(k, P), :])
                if (c * KS + k) % 2 == 0:
                    nc.vector.tensor_copy(w_sb[:, c, k], w_f32)
                else:
                    nc.scalar.copy(w_sb[:, c, k], w_f32)

        load_w(0)
        load_w(1)
        for t in range(2, N_TILES):
            transpose_tile(t)
            if t - 2 < 6:
                load_w(t)  # experts 2..7 behind tiles 2..7
        # (experts 2..7 all covered by the loop above since N_TILES-2 == 6)

        # --- Phase 3: scale + matmul-accumulate, experts in 4 groups of 2 so
        # PE only ever waits on a quarter of the w load; partial sums chain
        # through an SBUF accumulator per tile. ---
        GROUPS = 2
        CPG = N_CTRL // GROUPS
        accs = [None] * N_TILES
        for g in range(GROUPS):
            for t in range(N_TILES):
                # Fold routing weights in: xs[:, ci] = X^T * weights[:, c]
                # — one DVE op for the whole expert group (bf16 2x).
                xs = xspool.tile([P, CPG, KS, P], BF16)
                nc.vector.tensor_mul(
                    xs,
                    xts[t][:, None, :, :].to_broadcast([P, CPG, KS, P]),
                    wb[:, ts(g, CPG), None, ts(t, P)].to_broadcast(
                        [P, CPG, KS, P]
                    ),
                )

                out_ps = mm_psum.tile([P, D_OUT], F32, tag="acc")
                if g == 0:
                    # Bias rides in the first group's accumulation.
                    nc.tensor.matmul(
                        out_ps,
                        lhsT=wt_pad[:, ts(t, P)],
                        rhs=b_pad,
                        start=True,
                        stop=False,
                    )
                for ci in range(CPG):
                    c = g * CPG + ci
                    for k in range(KS):
                        nc.tensor.matmul(
                            out_ps,
                            lhsT=xs[:, ci, k, :],
                            rhs=w_sb[:, c, k, :],
                            start=(g != 0 and ci == 0 and k == 0),
                            stop=(ci == CPG - 1 and k == KS - 1),
                        )

                if g == 0:
                    acc = opool.tile([P, D_OUT], F32, tag="acc_sb", bufs=N_TILES)
                    nc.scalar.copy(acc, out_ps)
                    accs[t] = acc
                elif g < GROUPS - 1:
                    # acc += psum (in-place on DVE)
                    nc.vector.tensor_add(accs[t], out_ps, accs[t])
                else:
                    o_sb = opool.tile([P, D_OUT], F32, tag="o_sb")
                    nc.vector.tensor_add(o_sb, out_ps, accs[t])
                    nc.sync.dma_start(o_d[ts(t, P), :], o_sb)


def _split_multi_waits(bir: dict) -> dict:
    """The walrus build in this container supports at most ONE sync-wait per
    instruction ("Too many sync wait commands" at codegen otherwise).  Tile's
    scheduler freely attaches several.  Split: keep the last wait on the
    instruction and hoist the others onto standalone same-engine
    EventSemaphore instructions inserted immediately before it — identical
    semantics (the engine blocks at the same program point)."""
    ctr = 0
    for func in bir["functions"]:
        for bb in func["blocks"]:
            new_insts = []
            for inst in bb["instructions"]:
                si = inst.get("sync_info")
                waits = si.get("on_wait") if si else None
                if waits and len(waits) > 1:
                    for w in waits[:-1]:
                        ctr += 1
                        new_insts.append(
                            {
                                "debug": inst.get("debug", 0),
                                "engine": inst["engine"],
                                "ins": [],
                                "outs": [],
                                "name": f"{inst['name']}-wsplit{ctr}",
                                "opcode": "EventSemaphore",
                                "sync_info": {"on_update": [], "on_wait": [w]},
                            }
                        )
                    si["on_wait"] = [waits[-1]]
                new_insts.append(inst)
            bb["instructions"] = new_insts
    return bir


_ORIG_TO_JSON_BYTES = bass.Bass.to_json_bytes


def _patched_to_json_bytes(self) -> bytes:
    bir = json.loads(_ORIG_TO_JSON_BYTES(self))
    _split_multi_waits(bir)
    return json.dumps(bir).encode()


_NC_CACHE = []


def _build() -> bass.Bass:
    if _NC_CACHE:
        return _NC_CACHE[0]
    nc = bass.Bass(
        "TRN2",
        target_bir_lowering=False,
        debug=False,
        enable_asserts=False,
        num_devices=N_CORES,
    )
    x_d = nc.dram_tensor("x_in", [B_SHARD, D_IN], F32, kind="ExternalInput").ap()
    wt_d = nc.dram_tensor("wt_in", [B_SHARD, N_CTRL], F32, kind="ExternalInput").ap()
    w_d = nc.dram_tensor("w_in", [N_CTRL, D_IN, D_OUT], F32, kind="ExternalInput").ap()
    b_d = nc.dram_tensor("b_in", [N_CTRL, D_OUT], F32, kind="ExternalInput").ap()
    o_d = nc.dram_tensor("out", [B_SHARD, D_OUT], F32, kind="ExternalOutput").ap()
    with tile.TileContext(nc) as tc:
        with tc.tile_pool(name="global_const", bufs=1) as gconst:
            identity, e_pad = _consts(nc, gconst)
            _body(nc, tc, x_d, wt_d, w_d, b_d, o_d, identity, e_pad)
    nc.to_json_bytes = types.MethodType(_patched_to_json_bytes, nc)
    _NC_CACHE.append(nc)
    return nc


def kernel(inputs, weights, w, b, _trace=False):
    nc = _build()
    inputs = np.ascontiguousarray(inputs, dtype=np.float32)
    weights = np.ascontiguousarray(weights, dtype=np.float32)
    w = np.ascontiguousarray(w, dtype=np.float32)
    b = np.ascontiguousarray(b, dtype=np.float32)

    in_maps = []
    for i in range(N_CORES):
        sl = slice(i * B_SHARD, (i + 1) * B_SHARD)
        in_maps.append(
            {
                "x_in": inputs[sl],
                "wt_in": weights[sl],
                "w_in": w,
                "b_in": b,
            }
        )
    res = run_bass_kernel_spmd(
        nc, in_maps, core_ids=list(range(N_CORES)), trace=_trace
    )
    out = np.concatenate([r["out"] for r in res.results], axis=0)
    if _trace:
        return out, res
    return out
